# revision 1
# baseline (speedup 1.0000x reference)
"""GCN v3: core-scope src dedup. Each core gathers every unique src row ONCE
(~20.2k idx vs 81920 edges), then one wide PSUM accumulator [128f, 2560n]
takes S-chunk matmuls with host-built multi-hot scatter matrices [128, 2560].
Pool (Q7 gather) drops to ~175us; S streaming (~100MB/core) becomes the wall.
"""

import numpy as np

N_NODES = 20000
D = 128
N_CORES = 8
TILE2 = 512
N_PAD = 20480
NPC = N_PAD // N_CORES             # 2560
TPT = NPC // TILE2                 # 5 epilogue tiles per core

_prog_cache = {}


def _build_program3(UC):
    import concourse.mybir as mybir
    from concourse import bacc
    from concourse.tile import TileContext

    dt = mybir.dt
    GCH = 8
    NG = (UC + GCH - 1) // GCH     # gather groups
    nc = bacc.Bacc()

    h16 = nc.declare_dram_parameter("h16", [N_NODES, D], dt.float16, isOutput=False)
    hT = nc.declare_dram_parameter("hT", [D, NPC], dt.float32, isOutput=False)
    idx = nc.declare_dram_parameter("idx", [128, UC * 8], dt.int16, isOutput=False)
    smat = nc.declare_dram_parameter("smat", [128, UC * NPC], dt.float16, isOutput=False)
    wselfT = nc.declare_dram_parameter("wselfT", [D, D], dt.float32, isOutput=False)
    wneiT = nc.declare_dram_parameter("wneiT", [D, D], dt.float32, isOutput=False)
    bself = nc.declare_dram_parameter("bself", [D, 1], dt.float32, isOutput=False)
    outT = nc.declare_dram_parameter("outT", [D, NPC], dt.float32, isOutput=True)

    with (
        TileContext(nc) as tc,
        tc.tile_pool(name="const", bufs=1) as cpool,
        tc.tile_pool(name="gather", bufs=1) as gpool,
        tc.tile_pool(name="sel", bufs=4) as spool,
        tc.tile_pool(name="agg", bufs=2) as apool,
        tc.tile_pool(name="res", bufs=2) as opool,
        tc.tile_pool(name="pagg", bufs=1, space="PSUM") as pagg,
        tc.tile_pool(name="pout", bufs=2, space="PSUM") as pout,
    ):
        hT_sb = cpool.tile([D, NPC], dt.float32)
        nc.sync.dma_start(out=hT_sb[:], in_=hT[:])
        idx_sb = cpool.tile([128, UC * 8], dt.int16)
        nc.sync.dma_start(out=idx_sb[:], in_=idx[:])
        wselfT_sb = cpool.tile([D, D], dt.float32)
        nc.sync.dma_start(out=wselfT_sb[:], in_=wselfT[:])
        wneiT_sb = cpool.tile([D, D], dt.float32)
        nc.sync.dma_start(out=wneiT_sb[:], in_=wneiT[:])
        bself_sb = cpool.tile([D, 1], dt.float32)
        nc.sync.dma_start(out=bself_sb[:], in_=bself[:])

        # per-group gather tiles (all stay live through the aggregation)
        gtiles = []
        for gi in range(NG):
            nch = min(GCH, UC - gi * GCH)
            g = gpool.tile([128, GCH * 128], dt.float16, tag=f"g{gi}")
            nc.gpsimd.dma_gather(
                g[:, : nch * 128].rearrange("p (c e) -> p c e", e=128),
                h16[:, :],
                idx_sb[:, gi * GCH * 8 : (gi * GCH + nch) * 8],
                nch * 128,
                nch * 128,
                128,
            )
            gtiles.append(g)

        pa = pagg.tile([128, NPC], dt.float32)
        for c in range(UC):
            s = spool.tile([128, NPC], dt.float16)
            nc.sync.dma_start(out=s[:], in_=smat[:, c * NPC : (c + 1) * NPC])
            g = gtiles[c // GCH]
            lhs = g[:, (c % GCH) * 128 : (c % GCH + 1) * 128]
            for k in range(TPT):
                nc.tensor.matmul(
                    out=pa[:, k * TILE2 : (k + 1) * TILE2],
                    lhsT=lhs,
                    rhs=s[:, k * TILE2 : (k + 1) * TILE2],
                    start=(c == 0),
                    stop=(c == UC - 1),
                )

        for k in range(TPT):
            aggT = apool.tile([128, TILE2], dt.float32)
            nc.vector.tensor_copy(out=aggT[:], in_=pa[:, k * TILE2 : (k + 1) * TILE2])
            po = pout.tile([128, TILE2], dt.float32)
            nc.tensor.matmul(
                out=po[:],
                lhsT=wselfT_sb[:],
                rhs=hT_sb[:, k * TILE2 : (k + 1) * TILE2],
                start=True,
                stop=False,
            )
            nc.tensor.matmul(
                out=po[:], lhsT=wneiT_sb[:], rhs=aggT[:], start=False, stop=True
            )
            o = opool.tile([128, TILE2], dt.float32)
            nc.scalar.activation(
                out=o[:],
                in_=po[:],
                func=mybir.ActivationFunctionType.Relu,
                bias=bself_sb[:, :1],
            )
            nc.sync.dma_start(out=outT[:, k * TILE2 : (k + 1) * TILE2], in_=o[:])

    nc.compile()
    return nc


def _balance_tiles3(dst):
    import heapq

    indeg = np.bincount(dst, minlength=N_NODES)
    order = np.argsort(-indeg, kind="stable")
    nt = N_PAD // TILE2
    heap = [(0, t) for t in range(nt)]
    heapq.heapify(heap)
    slots_used = np.zeros(nt, dtype=np.int64)
    slot_of_node = np.empty(N_NODES, dtype=np.int64)
    for n in order:
        cnt, t = heapq.heappop(heap)
        slot_of_node[n] = t * TILE2 + slots_used[t]
        slots_used[t] += 1
        if slots_used[t] < TILE2:
            heapq.heappush(heap, (cnt + int(indeg[n]), t))
    return slot_of_node


def _shard_edges3(edge_index, deg):
    src = np.asarray(edge_index[0], dtype=np.int64)
    dst = np.asarray(edge_index[1], dtype=np.int64)
    slot_of_node = _balance_tiles3(dst)
    dslot = slot_of_node[dst]
    core_id = dslot // NPC
    recip_nodes = 1.0 / np.maximum(deg.astype(np.float32), 1.0)

    uniq_list, s_list = [], []
    for cc in range(N_CORES):
        m = core_id == cc
        uniq, inv = np.unique(src[m], return_inverse=True)
        S = np.zeros((len(uniq), NPC), dtype=np.float32)
        np.add.at(S, (inv, dslot[m] - cc * NPC), recip_nodes[dst[m]])
        uniq_list.append(uniq)
        s_list.append(S)

    UC = max(1, int(np.ceil(max(len(u) for u in uniq_list) / 128)))
    cap = UC * 128
    per_core = []
    for cc in range(N_CORES):
        u = uniq_list[cc]
        idx_arr = np.zeros(cap, dtype=np.int16)
        idx_arr[: len(u)] = u.astype(np.int16)
        idxw = np.tile(idx_arr.reshape(cap // 16, 16).T, (8, 1)).astype(np.int16)
        s_pad = np.zeros((cap, NPC), dtype=np.float16)
        s_pad[: len(u)] = s_list[cc].astype(np.float16)
        sm = s_pad.reshape(UC, 128, NPC).transpose(1, 0, 2)
        sm = np.ascontiguousarray(sm).reshape(128, UC * NPC)
        per_core.append((np.ascontiguousarray(idxw), sm))
    return UC, per_core, slot_of_node


def kernel(h, edge_index, deg, w_self, b_self, w_nei):
    import os

    from concourse.bass_utils import run_bass_kernel_spmd

    h = np.asarray(h, dtype=np.float32)
    deg = np.asarray(deg, dtype=np.float32)

    UC, per_core, slot_of_node = _shard_edges3(edge_index, deg)

    h16 = np.ascontiguousarray(h.astype(np.float16))
    hT_pad = np.zeros((D, N_PAD), dtype=np.float32)
    hT_pad[:, slot_of_node] = h.T
    wselfT = np.ascontiguousarray(np.asarray(w_self, dtype=np.float32).T)
    wneiT = np.ascontiguousarray(np.asarray(w_nei, dtype=np.float32).T)
    b_col = np.ascontiguousarray(np.asarray(b_self, dtype=np.float32).reshape(D, 1))

    in_maps = []
    for cc in range(N_CORES):
        idxw, sm = per_core[cc]
        in_maps.append(
            {
                "h16": h16,
                "hT": np.ascontiguousarray(hT_pad[:, cc * NPC : (cc + 1) * NPC]),
                "idx": idxw,
                "smat": sm,
                "wselfT": wselfT,
                "wneiT": wneiT,
                "bself": b_col,
            }
        )

    if UC not in _prog_cache:
        _prog_cache[UC] = _build_program3(UC)
    nc = _prog_cache[UC]

    trace = bool(int(os.environ.get("GCN_TRACE", "0")))
    res = run_bass_kernel_spmd(nc, in_maps, core_ids=list(range(N_CORES)), trace=trace)
    kernel.last_results = res

    outT = np.concatenate([r["outT"] for r in res.results], axis=1)
    return np.ascontiguousarray(outT[:, slot_of_node].T, dtype=np.float32)



# revision 2
# speedup vs baseline: 2.3872x; 2.3872x over previous
"""GCN v4: dense fp8 aggregation matmul, no gather.

Replace the v3 dedup-gather + fp16 multi-hot scatter stream (~103MB/core +
167us GpSimd gather) with a dense per-core count matrix S8 [20480 src,
2560 dst] in fp8 (counts are small ints - exact in e4m3). aggT accumulates
as sum over 256-row chunk-pairs of h8_chunk^T @ S8_chunk using fp8
DoubleRow matmuls (0.5 cyc/row). recip(deg) is applied exactly in fp32 at
PSUM->SBUF copy time; the epilogue GEMMs run in bf16. HBM traffic drops
to ~59MB/core (S8 52.4MB dominates) -> DMA-roofline ~170us.
"""

import numpy as np

N_NODES = 20000
D = 128
N_CORES = 8
N_PAD = 20480
NPC = N_PAD // N_CORES            # 2560 dst slots per core
TILE2 = 512
TPT = NPC // TILE2                # 5 psum column windows
NCP = N_PAD // 256                # 80 src chunk-pairs (256 rows each)

_prog_cache = {}


def _build_program4(double_row=True):
    import concourse.mybir as mybir
    from concourse import bacc
    from concourse.tile import TileContext

    dt = mybir.dt
    nc = bacc.Bacc()

    h8 = nc.declare_dram_parameter("h8", [128, N_PAD], dt.float8e4, isOutput=False)
    smat = nc.declare_dram_parameter(
        "smat", [128, NCP * 2 * NPC], dt.float8e4, isOutput=False
    )
    hT = nc.declare_dram_parameter("hT", [D, NPC], dt.bfloat16, isOutput=False)
    recip = nc.declare_dram_parameter("recip", [128, NPC], dt.float32, isOutput=False)
    wselfT = nc.declare_dram_parameter("wselfT", [D, D], dt.bfloat16, isOutput=False)
    wneiT = nc.declare_dram_parameter("wneiT", [D, D], dt.bfloat16, isOutput=False)
    bself = nc.declare_dram_parameter("bself", [D, 1], dt.float32, isOutput=False)
    outT = nc.declare_dram_parameter("outT", [D, NPC], dt.float32, isOutput=True)

    with (
        TileContext(nc) as tc,
        tc.tile_pool(name="const", bufs=1) as cpool,
        tc.tile_pool(name="sel", bufs=6) as spool,
        tc.tile_pool(name="agg", bufs=2) as apool,
        tc.tile_pool(name="res", bufs=2) as opool,
        tc.tile_pool(name="pagg", bufs=1, space="PSUM") as pagg,
        tc.tile_pool(name="pout", bufs=2, space="PSUM") as pout,
    ):
        h8_sb = cpool.tile([128, N_PAD], dt.float8e4)
        nc.sync.dma_start(out=h8_sb[:], in_=h8[:])
        hT_sb = cpool.tile([D, NPC], dt.bfloat16)
        nc.sync.dma_start(out=hT_sb[:], in_=hT[:])
        recip_sb = cpool.tile([128, NPC], dt.float32)
        nc.sync.dma_start(out=recip_sb[:], in_=recip[:])
        wselfT_sb = cpool.tile([D, D], dt.bfloat16)
        nc.sync.dma_start(out=wselfT_sb[:], in_=wselfT[:])
        wneiT_sb = cpool.tile([D, D], dt.bfloat16)
        nc.sync.dma_start(out=wneiT_sb[:], in_=wneiT[:])
        bself_sb = cpool.tile([D, 1], dt.float32)
        nc.sync.dma_start(out=bself_sb[:], in_=bself[:])

        # [128, cp, 2, 128]: row (cp*256 + i*128 + p) of padded h, fp8
        h8r = h8_sb.rearrange("p (cp two m) -> p cp two m", two=2, m=128)

        pa = pagg.tile([128, NPC], dt.float32)
        for cp in range(NCP):
            s = spool.tile([128, 2 * NPC], dt.float8e4)
            nc.sync.dma_start(out=s[:], in_=smat[:, cp * 2 * NPC : (cp + 1) * 2 * NPC])
            sr = s.rearrange("p (two n) -> p two n", two=2)
            for k in range(TPT):
                if double_row:
                    nc.tensor.matmul(
                        out=pa[:, k * TILE2 : (k + 1) * TILE2],
                        lhsT=h8r[:, cp, :, :],
                        rhs=sr[:, :, k * TILE2 : (k + 1) * TILE2],
                        start=(cp == 0),
                        stop=(cp == NCP - 1),
                        perf_mode=mybir.MatmulPerfMode.DoubleRow,
                    )
                else:
                    for i in range(2):
                        nc.tensor.matmul(
                            out=pa[:, k * TILE2 : (k + 1) * TILE2],
                            lhsT=h8r[:, cp, i, :],
                            rhs=sr[:, i, k * TILE2 : (k + 1) * TILE2],
                            start=(cp == 0 and i == 0),
                            stop=(cp == NCP - 1 and i == 1),
                        )

        for k in range(TPT):
            sl = slice(k * TILE2, (k + 1) * TILE2)
            aggT = apool.tile([128, TILE2], dt.bfloat16)
            nc.vector.tensor_mul(out=aggT[:], in0=pa[:, sl], in1=recip_sb[:, sl])
            po = pout.tile([128, TILE2], dt.float32, space="PSUM")
            nc.tensor.matmul(
                out=po[:], lhsT=wselfT_sb[:], rhs=hT_sb[:, sl], start=True, stop=False
            )
            nc.tensor.matmul(
                out=po[:], lhsT=wneiT_sb[:], rhs=aggT[:], start=False, stop=True
            )
            o = opool.tile([128, TILE2], dt.float32)
            nc.scalar.activation(
                out=o[:],
                in_=po[:],
                func=mybir.ActivationFunctionType.Relu,
                bias=bself_sb[:, :1],
            )
            nc.sync.dma_start(out=outT[:, sl], in_=o[:])

    nc.compile()
    return nc


def _host_prep(h, edge_index, deg):
    import ml_dtypes

    f8 = ml_dtypes.float8_e4m3
    bf16 = ml_dtypes.bfloat16

    src = np.asarray(edge_index[0], dtype=np.int64)
    dst = np.asarray(edge_index[1], dtype=np.int64)
    h = np.asarray(h, dtype=np.float32)
    deg = np.asarray(deg, dtype=np.float32)

    h_pad = np.zeros((N_PAD, D), np.float32)
    h_pad[:N_NODES] = h
    h8_flat = (
        h_pad.astype(f8).reshape(NCP, 2, 128, D).transpose(2, 0, 1, 3).reshape(128, -1)
    )
    h8_flat = np.ascontiguousarray(h8_flat)

    recip = np.zeros(N_PAD, np.float32)
    recip[:N_NODES] = 1.0 / np.maximum(deg, 1.0)

    lut = np.arange(256).astype(np.float32).astype(f8)

    core_of_dst = dst // NPC
    order = np.argsort(core_of_dst, kind="stable")
    src_s, dst_s = src[order], dst[order]
    bounds = np.searchsorted(core_of_dst[order], np.arange(N_CORES + 1))

    per_core = []
    for cc in range(N_CORES):
        lo, hi = bounds[cc], bounds[cc + 1]
        s_u8 = np.zeros((N_PAD, NPC), np.uint8)
        np.add.at(s_u8, (src_s[lo:hi], dst_s[lo:hi] - cc * NPC), 1)
        s8 = lut[s_u8]
        s8 = s8.reshape(NCP, 2, 128, NPC).transpose(2, 0, 1, 3).reshape(128, -1)
        per_core.append(np.ascontiguousarray(s8))

    hT_bf = np.ascontiguousarray(h_pad.T.astype(bf16))
    return h8_flat, per_core, recip, hT_bf


def kernel(h, edge_index, deg, w_self, b_self, w_nei):
    import os

    import ml_dtypes
    from concourse.bass_utils import run_bass_kernel_spmd

    bf16 = ml_dtypes.bfloat16

    h8_flat, per_core, recip, hT_bf = _host_prep(h, edge_index, deg)

    wselfT = np.ascontiguousarray(np.asarray(w_self, dtype=np.float32).T.astype(bf16))
    wneiT = np.ascontiguousarray(np.asarray(w_nei, dtype=np.float32).T.astype(bf16))
    b_col = np.ascontiguousarray(np.asarray(b_self, dtype=np.float32).reshape(D, 1))

    in_maps = []
    for cc in range(N_CORES):
        in_maps.append(
            {
                "h8": h8_flat,
                "smat": per_core[cc],
                "hT": np.ascontiguousarray(hT_bf[:, cc * NPC : (cc + 1) * NPC]),
                "recip": np.ascontiguousarray(
                    np.broadcast_to(recip[cc * NPC : (cc + 1) * NPC], (128, NPC))
                ),
                "wselfT": wselfT,
                "wneiT": wneiT,
                "bself": b_col,
            }
        )

    double_row = os.environ.get("GCN_NO_DR", "0") != "1"
    key = ("v4", double_row)
    if key not in _prog_cache:
        _prog_cache[key] = _build_program4(double_row)
    nc = _prog_cache[key]

    trace = bool(int(os.environ.get("GCN_TRACE", "0")))
    res = run_bass_kernel_spmd(nc, in_maps, core_ids=list(range(N_CORES)), trace=trace)
    kernel.last_results = res

    outT = np.concatenate([r["outT"] for r in res.results], axis=1)
    return np.ascontiguousarray(outT[:, :N_NODES].T, dtype=np.float32)
